# revision 1
# baseline (speedup 1.0000x reference)
"""Trainium2 Bass kernel for 3-layer CuGraphSAGE on a fanout-8 sampled tree.

The sampled graph produced by fanout-based neighbor sampling is a forest of
B=4096 independent trees (children of parent p are rows [4096+8p, 4096+8p+8)).
We shard by seed block: core c gets 512 seeds plus their full 3-hop subtrees
(4 contiguous row blocks of x, exactly 1/8 of all rows, zero halo).

Per-core pipeline (all activations channel-major [128ch, rows] so the matmul
contraction dim is always the partition dim — no transposes on device):
  mean-aggregation = 8 accumulating matmuls with stride-8 rhs APs, the 1/8
  folded into the aggregation weight; self term = 1 more matmul into the same
  PSUM bank; bias+ReLU on ScalarE evicts PSUM->SBUF. h1/h2 live entirely in
  SBUF; only x is streamed from HBM (153.6 MB/core) and 2.25 MB stored.
"""

import os
import numpy as np

# ---------------------------------------------------------------- constants
N_CORES = 8
C = 128                       # channels
B = 4096                      # seeds
S = B // N_CORES              # 512 seeds per core
BLK = [512, 4096, 32768, 262144]          # per-core rows per hop
OFF = [0, 4096, 36864, 299008]            # global start row of each hop block
NLOC = sum(BLK)                           # 299520 local rows
NPAR0 = BLK[0] + BLK[1] + BLK[2]          # 37376 local layer-0 parents
NPAR1 = BLK[0] + BLK[1]                   # 4608 local layer-1 parents
PT = 512                                  # parents per PSUM tile
N_FULL = 2396160
E_FULL = 2392064
OUT_ROWS = 36864

TRACE = os.environ.get("GNN_TRACE", "0") == "1"
DTYPE = os.environ.get("GNN_DTYPE", "float32")
# aggregation path: "dve" = VectorE group-reduce + 1 matmul (best for f32,
# where matmul streams at 1/4 rate); "pe" = 8 accumulating matmuls with
# stride-8 rhs (best for 16-bit dtypes)
AGG = os.environ.get("GNN_AGG", "dve")
LAST_RESULT = None

_BASS_CACHE = {}


def _build_bass(dtype_str, agg):
    import concourse.mybir as mybir
    from concourse import bacc
    from concourse.tile import TileContext

    dt = getattr(mybir.dt, dtype_str)
    f32 = mybir.dt.float32
    Relu = mybir.ActivationFunctionType.Relu
    AxX = mybir.AxisListType.X

    # Bacc (not raw Bass): its compile() pipeline splits multi-sem sync
    # waits into event semaphores — TRN2 allows at most 1 wait/instruction.
    nc = bacc.Bacc()
    xT = nc.dram_tensor("xT", [C, NLOC], dt, kind="ExternalInput")
    # all six 128x128 weight blocks packed into one tensor -> one DMA ->
    # one semaphore lane (per-instruction sync-wait slots are scarce)
    wconsts = nc.dram_tensor("wconsts", [C, 6 * C], dt, kind="ExternalInput")
    bconsts = nc.dram_tensor("bconsts", [C, 3], f32, kind="ExternalInput")
    out = nc.dram_tensor("out", [C, NPAR1], f32, kind="ExternalOutput")
    WIDX = {k: i for i, k in
            enumerate(("w1a", "w1b", "w2a", "w2b", "w3a", "w3b"))}

    with TileContext(nc) as tc:
        with tc.tile_pool(name="const", bufs=1) as constp, \
             tc.tile_pool(name="keep", bufs=1) as keepp, \
             tc.tile_pool(name="cbuf", bufs=2) as cpool, \
             tc.tile_pool(name="dbuf", bufs=3) as dpool, \
             tc.tile_pool(name="hbuf", bufs=2) as hpool, \
             tc.tile_pool(name="obuf", bufs=2) as opool, \
             tc.tile_pool(name="aggbuf", bufs=4) as aggp, \
             tc.tile_pool(name="ps", bufs=6, space="PSUM") as pp:

            wtile = constp.tile([C, 6 * C], dt, name="wtile")
            nc.sync.dma_start(wtile[:, :], wconsts[:, :])
            btile = constp.tile([C, 3], f32, name="btile")
            nc.sync.dma_start(btile[:, :], bconsts[:, :])
            w = {k: wtile[:, C * i: C * (i + 1)] for k, i in WIDX.items()}
            bt = {f"b{i+1}": btile[:, i: i + 1] for i in range(3)}

            xA01 = keepp.tile([C, NPAR1], dt, tag="xA01")
            nc.sync.dma_start(xA01[:, :], xT[:, 0:NPAR1])
            h1self = keepp.tile([C, NPAR1], dt, tag="h1self")
            h2sb = keepp.tile([C, NPAR1], dt, tag="h2sb")

            def sage_tile(psum, wa, wb, children_ap, self_ap):
                # psum[o, p] = sum_e (W_a/8)[o,:] @ children[:, 8p+e]
                #            +  W_b[o,:] @ self[:, p]
                if agg == "pe":
                    cv = children_ap.rearrange("c (p e) -> c p e", e=8)
                    for e in range(8):
                        nc.tensor.matmul(psum, w[wa], cv[:, :, e],
                                         start=(e == 0), stop=False)
                else:
                    # group-sum the 8 siblings on VectorE (stride-1 inner
                    # reduce), then contract once on the TensorEngine --
                    # fp32 matmul streams at 1/4 rate, so 8x fewer matmuls
                    # wins even though DVE reduce is 1 elem/cycle/lane.
                    aggt = aggp.tile([C, PT], dt, tag="agg", name="aggt")
                    nc.vector.reduce_sum(
                        aggt[:, :],
                        children_ap.rearrange("c (p e) -> c p e", e=8),
                        axis=AxX)
                    nc.tensor.matmul(psum, w[wa], aggt[:, :],
                                     start=True, stop=False)
                nc.tensor.matmul(psum, w[wb], self_ap,
                                 start=False, stop=True)

            n_t = NPAR1 // PT                    # 9 outer tiles
            for t in range(n_t):
                # x rows [512+4096t, 512+4096(t+1)): children of layer-0
                # parents [512t, 512(t+1)) AND (for t>=1) self-features of
                # layer-0 parents [512+4096t, ...).
                Ct = cpool.tile([C, 8 * PT], dt, tag="C")
                nc.sync.dma_start(Ct[:, :],
                                  xT[:, S + 8 * PT * t: S + 8 * PT * (t + 1)])

                # layer-0 tile -> h1self[:, 512t:512(t+1)]
                ps0 = pp.tile([C, PT], f32, tag="ps")
                sage_tile(ps0, "w1a", "w1b", Ct[:, :],
                          xA01[:, PT * t: PT * (t + 1)])
                nc.scalar.activation(h1self[:, PT * t: PT * (t + 1)], ps0,
                                     Relu, bias=bt["b1"])

                if t == 0:
                    # parents [512, 4608) already land in h1self via the
                    # ps0 tiles t=1..8; layer-1 tile 0 is emitted after the
                    # loop reading h1self directly (no duplicate compute or
                    # duplicate HBM read of their children).
                    continue

                # 8 layer-0 tiles for parents [512+4096t, 512+4096(t+1))
                h1tmp = hpool.tile([C, 8 * PT], dt, tag="h1tmp")
                for u in range(8):
                    base = NPAR1 + 8 * PT * (8 * t + u)
                    D = dpool.tile([C, 8 * PT], dt, tag="D")
                    nc.sync.dma_start(D[:, :], xT[:, base: base + 8 * PT])
                    psu = pp.tile([C, PT], f32, tag="ps")
                    sage_tile(psu, "w1a", "w1b", D[:, :],
                              Ct[:, PT * u: PT * (u + 1)])
                    nc.scalar.activation(h1tmp[:, PT * u: PT * (u + 1)], psu,
                                         Relu, bias=bt["b1"])

                # layer-1 tile for parents [512t, 512(t+1)) -> h2
                ps1 = pp.tile([C, PT], f32, tag="ps")
                sage_tile(ps1, "w2a", "w2b", h1tmp[:, :],
                          h1self[:, PT * t: PT * (t + 1)])
                nc.scalar.activation(h2sb[:, PT * t: PT * (t + 1)], ps1,
                                     Relu, bias=bt["b2"])

            # layer-1 tile 0: children h1[512:4608) = h1self slice
            ps1z = pp.tile([C, PT], f32, tag="ps")
            sage_tile(ps1z, "w2a", "w2b", h1self[:, S:NPAR1],
                      h1self[:, 0:S])
            nc.scalar.activation(h2sb[:, 0:S], ps1z, Relu, bias=bt["b2"])

            # layer 2: parents [0, 512) aggregate h2[512:4608); rows
            # [512, 4608) have no in-edges (agg = 0) -> self term only.
            ps2 = pp.tile([C, PT], f32, tag="ps")
            sage_tile(ps2, "w3a", "w3b", h2sb[:, S:NPAR1], h2sb[:, 0:S])
            o0 = opool.tile([C, PT], f32, tag="o")
            nc.scalar.activation(o0[:, :], ps2, Relu, bias=bt["b3"])
            nc.sync.dma_start(out[:, 0:S], o0[:, :])
            for t in range(1, n_t):
                psn = pp.tile([C, PT], f32, tag="ps")
                nc.tensor.matmul(psn, w["w3b"],
                                 h2sb[:, PT * t: PT * (t + 1)],
                                 start=True, stop=True)
                on = opool.tile([C, PT], f32, tag="o")
                nc.scalar.activation(on[:, :], psn, Relu, bias=bt["b3"])
                nc.sync.dma_start(out[:, PT * t: PT * (t + 1)], on[:, :])

    nc.compile()
    return nc


def _get_bass(dtype_str, agg="dve"):
    key = (dtype_str, agg)
    if key not in _BASS_CACHE:
        _BASS_CACHE[key] = _build_bass(dtype_str, agg)
    return _BASS_CACHE[key]


def _edge_is_tree(edge):
    if edge.shape != (2, E_FULL):
        return False
    ar = np.arange(E_FULL, dtype=np.int64)
    return (np.array_equal(edge[0], (B + ar).astype(np.int32))
            and np.array_equal(edge[1], (ar // 8).astype(np.int32)))


def _fallback(x, edge, W1, b1, W2, b2, W3, b3):
    # General (structure-agnostic) CPU implementation; only used if the
    # inputs are not the fanout-8 tree this kernel is specialized for.
    sizes = [(N_FULL, E_FULL), (299008, 294912), (36864, 32768)]
    params = [(W1, b1), (W2, b2), (W3, b3)]
    x = x.astype(np.float32)
    for (n, e), (Wl, bl) in zip(sizes, params):
        src = edge[0, :e].astype(np.int64)
        dst = edge[1, :e].astype(np.int64)
        x = x[:n]
        agg = np.zeros((n, x.shape[1]), np.float32)
        np.add.at(agg, dst, x[src])
        deg = np.bincount(dst, minlength=n).astype(np.float32)
        agg /= np.maximum(deg, 1.0)[:, None]
        x = np.maximum(np.concatenate([agg, x], axis=1) @ Wl.T + bl, 0.0)
    return x


def kernel(**inputs):
    global LAST_RESULT
    x = np.asarray(inputs["x"])
    edge = np.asarray(inputs["edge"])
    W = [np.asarray(inputs[k], dtype=np.float32) for k in ("W1", "W2", "W3")]
    bias = [np.asarray(inputs[k], dtype=np.float32) for k in ("b1", "b2", "b3")]

    if x.shape != (N_FULL, C) or not _edge_is_tree(edge):
        return _fallback(x, edge, W[0], bias[0], W[1], bias[1], W[2], bias[2])

    from concourse.bass_utils import run_bass_kernel_spmd

    if DTYPE == "bfloat16":
        np_dt = _bf16()
    else:
        np_dt = {"float32": np.float32, "float32r": np.float32,
                 "float16": np.float16}[DTYPE]
    x = np.ascontiguousarray(x, dtype=np.float32)

    wblocks = []
    for li in range(3):
        wblocks.append((W[li][:, :C] / 8.0).T)     # agg part, mean folded in
        wblocks.append(W[li][:, C:].T)             # self part
    wconsts = np.ascontiguousarray(np.concatenate(wblocks, axis=1)).astype(np_dt)
    bconsts = np.ascontiguousarray(np.stack(bias, axis=1))      # [128, 3] f32

    in_maps = []
    for c in range(N_CORES):
        xloc = np.concatenate(
            [x[OFF[h] + BLK[h] * c: OFF[h] + BLK[h] * (c + 1)] for h in range(4)],
            axis=0)
        xTc = np.ascontiguousarray(xloc.T).astype(np_dt, copy=False)
        in_maps.append({"xT": xTc, "wconsts": wconsts, "bconsts": bconsts})

    nc = _get_bass(DTYPE, AGG)
    res = run_bass_kernel_spmd(nc, in_maps, list(range(N_CORES)), trace=TRACE)
    LAST_RESULT = res

    out = np.empty((OUT_ROWS, C), np.float32)
    for c in range(N_CORES):
        oc = np.asarray(res.results[c]["out"])
        out[S * c: S * (c + 1)] = oc[:, :S].T
        out[B + 8 * S * c: B + 8 * S * (c + 1)] = oc[:, S:].T
    return out


def _bf16():
    import ml_dtypes
    return ml_dtypes.bfloat16



# revision 5
# speedup vs baseline: 1.6388x; 1.6388x over previous
"""Trainium2 Bass kernel for 3-layer CuGraphSAGE on a fanout-8 sampled tree.

The sampled graph produced by fanout-based neighbor sampling is a forest of
B=4096 independent trees (children of parent p are rows [4096+8p, 4096+8p+8)).
We shard by seed block: core c gets 512 seeds plus their full 3-hop subtrees
(4 contiguous row blocks of x, exactly 1/8 of all rows, zero halo).

Per-core pipeline (all activations channel-major [128ch, rows] so the matmul
contraction dim is always the partition dim — no transposes on device):
  hop3 (87.5% of bytes) is stored bf16 AND de-interleaved per 512-parent
  group on the host (col e*512+p = child e of parent p), so the mean-
  aggregation is 8 accumulating matmuls with CONTIGUOUS rhs slices (stride-8
  APs stream ~5x slower on the PE). The small hop1/hop2 and layer-1/2
  aggregations run as VectorE reduce_sum (DVE is otherwise idle) followed by
  one matmul. The 1/8 mean is folded into the aggregation weight; self term
  is one more matmul into the same PSUM bank; bias+ReLU on ScalarE evicts
  PSUM->SBUF. h1/h2 live entirely in SBUF; only x streams from HBM.
"""

import os
import numpy as np

# ---------------------------------------------------------------- constants
N_CORES = 8
C = 128                       # channels
B = 4096                      # seeds
S = B // N_CORES              # 512 seeds per core
BLK = [512, 4096, 32768, 262144]          # per-core rows per hop
OFF = [0, 4096, 36864, 299008]            # global start row of each hop block
NLOC = sum(BLK)                           # 299520 local rows
NPAR0 = BLK[0] + BLK[1] + BLK[2]          # 37376 local layer-0 parents
NPAR1 = BLK[0] + BLK[1]                   # 4608 local layer-1 parents
PT = 512                                  # parents per PSUM tile
N_FULL = 2396160
E_FULL = 2392064
OUT_ROWS = 36864

TRACE = os.environ.get("GNN_TRACE", "0") == "1"
DTYPE = os.environ.get("GNN_DTYPE", "bfloat16")
LAST_RESULT = None

_BASS_CACHE = {}


def _build_bass(dtype_str):
    import concourse.mybir as mybir
    from concourse import bacc
    from concourse.tile import TileContext

    dt = getattr(mybir.dt, dtype_str)
    f32 = mybir.dt.float32
    Relu = mybir.ActivationFunctionType.Relu
    AxX = mybir.AxisListType.X

    # Bacc (not raw Bass): its compile() pipeline splits multi-sem sync
    # waits into event semaphores — TRN2 allows at most 1 wait/instruction.
    nc = bacc.Bacc()
    xT = nc.dram_tensor("xT", [C, NLOC], dt, kind="ExternalInput")
    # all six 128x128 weight blocks packed into one tensor -> one DMA ->
    # one semaphore lane (per-instruction sync-wait slots are scarce)
    wconsts = nc.dram_tensor("wconsts", [C, 6 * C], dt, kind="ExternalInput")
    bconsts = nc.dram_tensor("bconsts", [C, 3], f32, kind="ExternalInput")
    out = nc.dram_tensor("out", [C, NPAR1], f32, kind="ExternalOutput")
    WIDX = {k: i for i, k in
            enumerate(("w1a", "w1b", "w2a", "w2b", "w3a", "w3b"))}

    with TileContext(nc) as tc:
        with tc.tile_pool(name="const", bufs=1) as constp, \
             tc.tile_pool(name="keep", bufs=1) as keepp, \
             tc.tile_pool(name="dbuf", bufs=3) as dpool, \
             tc.tile_pool(name="hbuf", bufs=2) as hpool, \
             tc.tile_pool(name="obuf", bufs=2) as opool, \
             tc.tile_pool(name="aggbuf", bufs=4) as aggp, \
             tc.tile_pool(name="ps", bufs=6, space="PSUM") as pp:

            wtile = constp.tile([C, 6 * C], dt, name="wtile")
            nc.sync.dma_start(wtile[:, :], wconsts[:, :])
            btile = constp.tile([C, 3], f32, name="btile")
            nc.sync.dma_start(btile[:, :], bconsts[:, :])
            w = {k: wtile[:, C * i: C * (i + 1)] for k, i in WIDX.items()}
            bt = {f"b{i+1}": btile[:, i: i + 1] for i in range(3)}

            xA01 = keepp.tile([C, NPAR1], dt, tag="xA01")
            nc.sync.dma_start(xA01[:, :], xT[:, 0:NPAR1])
            h1self = keepp.tile([C, NPAR1], dt, tag="h1self")
            h2sb = keepp.tile([C, NPAR1], dt, tag="h2sb")

            def sage_dve(psum, wa, wb, children_ap, self_ap):
                # psum[o, p] = sum_e (W_a/8)[o,:] @ children[:, 8p+e]
                #            +  W_b[o,:] @ self[:, p]
                # children in natural node order -> group-sum the 8 siblings
                # on VectorE (1x-mode reduce, but DVE is idle), contract once.
                aggt = aggp.tile([C, PT], dt, tag="agg", name="aggt")
                with nc.allow_low_precision(
                        reason="8-way sibling sum is fp32 internal on DVE; "
                               "bf16 rounding of the sum is within tolerance"):
                    nc.vector.reduce_sum(
                        aggt[:, :],
                        children_ap.rearrange("c (p e) -> c p e", e=8),
                        axis=AxX)
                nc.tensor.matmul(psum, w[wa], aggt[:, :],
                                 start=True, stop=False)
                nc.tensor.matmul(psum, w[wb], self_ap,
                                 start=False, stop=True)

            def sage_pe(psum, wa, wb, children_ap, self_ap):
                # children de-interleaved on host: col e*PT + p = child e of
                # parent p -> 8 accumulating matmuls, each rhs contiguous.
                for e in range(8):
                    nc.tensor.matmul(psum, w[wa],
                                     children_ap[:, PT * e: PT * (e + 1)],
                                     start=(e == 0), stop=False)
                nc.tensor.matmul(psum, w[wb], self_ap,
                                 start=False, stop=True)

            n_t = NPAR1 // PT                    # 9 outer tiles
            for t in range(n_t):
                # layer-0 tile for parents [512t, 512(t+1)) (hop0+hop1):
                # children are x rows [512+4096t, ...) in natural order.
                if t == 0:
                    ch0 = xA01[:, S:NPAR1]       # hop1 = children of seeds
                else:
                    ch0 = dpool.tile([C, 8 * PT], dt, tag="D", name="h1ch")
                    nc.sync.dma_start(
                        ch0[:, :],
                        xT[:, S + 8 * PT * t: S + 8 * PT * (t + 1)])
                ps0 = pp.tile([C, PT], f32, tag="ps")
                sage_dve(ps0, "w1a", "w1b", ch0,
                         xA01[:, PT * t: PT * (t + 1)])
                nc.scalar.activation(h1self[:, PT * t: PT * (t + 1)], ps0,
                                     Relu, bias=bt["b1"])

                if t == 0:
                    # parents [512, 4608) land in h1self via tiles t=1..8;
                    # layer-1 tile 0 is emitted after the loop from h1self.
                    continue

                # 8 layer-0 tiles for hop2 parents [512+4096t, 512+4096(t+1))
                # children ⊂ hop3: bf16, host de-interleaved per 512-group.
                h1tmp = hpool.tile([C, 8 * PT], dt, tag="h1tmp")
                for u in range(8):
                    base = NPAR1 + 8 * PT * (8 * t + u)
                    D = dpool.tile([C, 8 * PT], dt, tag="D")
                    nc.sync.dma_start(D[:, :], xT[:, base: base + 8 * PT])
                    psu = pp.tile([C, PT], f32, tag="ps")
                    sage_pe(psu, "w1a", "w1b", D[:, :],
                            ch0[:, PT * u: PT * (u + 1)])
                    nc.scalar.activation(h1tmp[:, PT * u: PT * (u + 1)], psu,
                                         Relu, bias=bt["b1"])

                # layer-1 tile for parents [512t, 512(t+1)) -> h2
                ps1 = pp.tile([C, PT], f32, tag="ps")
                sage_dve(ps1, "w2a", "w2b", h1tmp[:, :],
                         h1self[:, PT * t: PT * (t + 1)])
                nc.scalar.activation(h2sb[:, PT * t: PT * (t + 1)], ps1,
                                     Relu, bias=bt["b2"])

            # layer-1 tile 0: children h1[512:4608) = h1self slice
            ps1z = pp.tile([C, PT], f32, tag="ps")
            sage_dve(ps1z, "w2a", "w2b", h1self[:, S:NPAR1],
                     h1self[:, 0:S])
            nc.scalar.activation(h2sb[:, 0:S], ps1z, Relu, bias=bt["b2"])

            # layer 2: parents [0, 512) aggregate h2[512:4608); rows
            # [512, 4608) have no in-edges (agg = 0) -> self term only.
            ps2 = pp.tile([C, PT], f32, tag="ps")
            sage_dve(ps2, "w3a", "w3b", h2sb[:, S:NPAR1], h2sb[:, 0:S])
            o0 = opool.tile([C, PT], f32, tag="o")
            nc.scalar.activation(o0[:, :], ps2, Relu, bias=bt["b3"])
            nc.sync.dma_start(out[:, 0:S], o0[:, :])
            for t in range(1, n_t):
                psn = pp.tile([C, PT], f32, tag="ps")
                nc.tensor.matmul(psn, w["w3b"],
                                 h2sb[:, PT * t: PT * (t + 1)],
                                 start=True, stop=True)
                on = opool.tile([C, PT], f32, tag="o")
                nc.scalar.activation(on[:, :], psn, Relu, bias=bt["b3"])
                nc.sync.dma_start(out[:, PT * t: PT * (t + 1)], on[:, :])

    nc.compile()
    return nc


def _get_bass(dtype_str):
    if dtype_str not in _BASS_CACHE:
        _BASS_CACHE[dtype_str] = _build_bass(dtype_str)
    return _BASS_CACHE[dtype_str]


def _edge_is_tree(edge):
    if edge.shape != (2, E_FULL):
        return False
    ar = np.arange(E_FULL, dtype=np.int64)
    return (np.array_equal(edge[0], (B + ar).astype(np.int32))
            and np.array_equal(edge[1], (ar // 8).astype(np.int32)))


def _fallback(x, edge, W1, b1, W2, b2, W3, b3):
    # General (structure-agnostic) CPU implementation; only used if the
    # inputs are not the fanout-8 tree this kernel is specialized for.
    sizes = [(N_FULL, E_FULL), (299008, 294912), (36864, 32768)]
    params = [(W1, b1), (W2, b2), (W3, b3)]
    x = x.astype(np.float32)
    for (n, e), (Wl, bl) in zip(sizes, params):
        src = edge[0, :e].astype(np.int64)
        dst = edge[1, :e].astype(np.int64)
        x = x[:n]
        agg = np.zeros((n, x.shape[1]), np.float32)
        np.add.at(agg, dst, x[src])
        deg = np.bincount(dst, minlength=n).astype(np.float32)
        agg /= np.maximum(deg, 1.0)[:, None]
        x = np.maximum(np.concatenate([agg, x], axis=1) @ Wl.T + bl, 0.0)
    return x


def _np_dtype(dtype_str):
    if dtype_str == "bfloat16":
        import ml_dtypes
        return ml_dtypes.bfloat16
    return {"float32": np.float32, "float32r": np.float32,
            "float16": np.float16}[dtype_str]


def kernel(**inputs):
    global LAST_RESULT
    x = np.asarray(inputs["x"])
    edge = np.asarray(inputs["edge"])
    W = [np.asarray(inputs[k], dtype=np.float32) for k in ("W1", "W2", "W3")]
    bias = [np.asarray(inputs[k], dtype=np.float32) for k in ("b1", "b2", "b3")]

    if x.shape != (N_FULL, C) or not _edge_is_tree(edge):
        return _fallback(x, edge, W[0], bias[0], W[1], bias[1], W[2], bias[2])

    from concourse.bass_utils import run_bass_kernel_spmd

    np_dt = _np_dtype(DTYPE)
    x = np.ascontiguousarray(x, dtype=np.float32)

    wblocks = []
    for li in range(3):
        wblocks.append((W[li][:, :C] / 8.0).T)     # agg part, mean folded in
        wblocks.append(W[li][:, C:].T)             # self part
    wconsts = np.ascontiguousarray(np.concatenate(wblocks, axis=1)).astype(np_dt)
    bconsts = np.ascontiguousarray(np.stack(bias, axis=1))      # [128, 3] f32

    in_maps = []
    for c in range(N_CORES):
        xloc = [x[OFF[h] + BLK[h] * c: OFF[h] + BLK[h] * (c + 1)]
                for h in range(4)]
        # de-interleave hop3 per 512-parent group: within each 4096-row
        # chunk, row e*512 + p  <-  child e of parent p (old row 8p + e)
        x3 = xloc[3].reshape(-1, PT, 8, C).transpose(0, 2, 1, 3).reshape(-1, C)
        xloc = np.concatenate(xloc[:3] + [x3], axis=0)
        xTc = np.ascontiguousarray(xloc.T).astype(np_dt, copy=False)
        in_maps.append({"xT": xTc, "wconsts": wconsts, "bconsts": bconsts})

    nc = _get_bass(DTYPE)
    res = run_bass_kernel_spmd(nc, in_maps, list(range(N_CORES)), trace=TRACE)
    LAST_RESULT = res

    out = np.empty((OUT_ROWS, C), np.float32)
    for c in range(N_CORES):
        oc = np.asarray(res.results[c]["out"])
        out[S * c: S * (c + 1)] = oc[:, :S].T
        out[B + 8 * S * c: B + 8 * S * (c + 1)] = oc[:, S:].T
    return out


# revision 6
# speedup vs baseline: 2.2941x; 1.3998x over previous
"""Trainium2 Bass kernel for 3-layer CuGraphSAGE on a fanout-8 sampled tree.

The sampled graph produced by fanout-based neighbor sampling is a forest of
B=4096 independent trees (children of parent p are rows [4096+8p, 4096+8p+8)).
We shard by seed block: core c gets 512 seeds plus their full 3-hop subtrees
(4 contiguous row blocks of x, exactly 1/8 of all rows, zero halo).

Per-core pipeline (all activations channel-major [128ch, rows] so the matmul
contraction dim is always the partition dim — no transposes on device):
  hop3 (87.5% of bytes, only ever aggregated) streams as fp8_e4m3 — the
  8-way mean dilutes its 3.6% quantization RMS to ~1e-3 of the final output.
  It is de-interleaved per 512-parent group on the host (col e*512+p =
  child e of parent p), so the mean-aggregation is 8 accumulating matmuls
  with CONTIGUOUS fp8 rhs slices against the bf16 aggregation weight
  (stride-8 APs stream ~5x slower on the PE; mixed fp8xbf16 is native).
  hops 0-2 (37376 rows) load once as a single resident bf16 tile serving
  all self terms and the hop1/hop2 aggregations; those small aggregations
  plus layers 1-2 run as VectorE reduce_sum (DVE is otherwise idle)
  followed by one matmul. The 1/8 mean is folded into the aggregation
  weight; bias+ReLU on ScalarE evicts PSUM->SBUF in bf16. h1/h2 live
  entirely in SBUF; the bf16 output is upcast to fp32 on the host.
"""

import os
import numpy as np

# ---------------------------------------------------------------- constants
N_CORES = 8
C = 128                       # channels
B = 4096                      # seeds
S = B // N_CORES              # 512 seeds per core
BLK = [512, 4096, 32768, 262144]          # per-core rows per hop
OFF = [0, 4096, 36864, 299008]            # global start row of each hop block
NPAR0 = BLK[0] + BLK[1] + BLK[2]          # 37376 local layer-0 parents
NPAR1 = BLK[0] + BLK[1]                   # 4608 local layer-1 parents
N3 = BLK[3]                               # 262144 local hop3 rows
PT = 512                                  # parents per PSUM tile
N_FULL = 2396160
E_FULL = 2392064
OUT_ROWS = 36864

TRACE = os.environ.get("GNN_TRACE", "0") == "1"
LAST_RESULT = None

_BASS_CACHE = {}


def _build_bass():
    import concourse.mybir as mybir
    from concourse import bacc
    from concourse.tile import TileContext

    bf16 = mybir.dt.bfloat16
    fp8 = mybir.dt.float8e4
    f32 = mybir.dt.float32
    Relu = mybir.ActivationFunctionType.Relu
    AxX = mybir.AxisListType.X

    # Bacc (not raw Bass): its compile() pipeline splits multi-sem sync
    # waits into event semaphores — TRN2 allows at most 1 wait/instruction.
    nc = bacc.Bacc()
    xA = nc.dram_tensor("xA", [C, NPAR0], bf16, kind="ExternalInput")
    x3 = nc.dram_tensor("x3", [C, N3], fp8, kind="ExternalInput")
    # all six 128x128 weight blocks packed into one tensor -> one DMA ->
    # one semaphore lane (per-instruction sync-wait slots are scarce)
    wconsts = nc.dram_tensor("wconsts", [C, 6 * C], bf16, kind="ExternalInput")
    bconsts = nc.dram_tensor("bconsts", [C, 3], f32, kind="ExternalInput")
    out = nc.dram_tensor("out", [C, NPAR1], bf16, kind="ExternalOutput")
    WIDX = {k: i for i, k in
            enumerate(("w1a", "w1b", "w2a", "w2b", "w3a", "w3b"))}

    with TileContext(nc) as tc:
        with tc.tile_pool(name="const", bufs=1) as constp, \
             tc.tile_pool(name="keep", bufs=1) as keepp, \
             tc.tile_pool(name="dbuf", bufs=2) as dpool, \
             tc.tile_pool(name="hbuf", bufs=2) as hpool, \
             tc.tile_pool(name="obuf", bufs=2) as opool, \
             tc.tile_pool(name="aggbuf", bufs=4) as aggp, \
             tc.tile_pool(name="ps", bufs=6, space="PSUM") as pp:

            wtile = constp.tile([C, 6 * C], bf16, name="wtile")
            nc.sync.dma_start(wtile[:, :], wconsts[:, :])
            btile = constp.tile([C, 3], f32, name="btile")
            nc.sync.dma_start(btile[:, :], bconsts[:, :])
            w = {k: wtile[:, C * i: C * (i + 1)] for k, i in WIDX.items()}
            bt = {f"b{i+1}": btile[:, i: i + 1] for i in range(3)}

            # hops 0-2: resident for the whole kernel (75 KiB/partition)
            xAt = keepp.tile([C, NPAR0], bf16, tag="xAt")
            nc.sync.dma_start(xAt[:, :], xA[:, :])
            h1self = keepp.tile([C, NPAR1], bf16, tag="h1self")
            h2sb = keepp.tile([C, NPAR1], bf16, tag="h2sb")

            def sage_dve(psum, wa, wb, children_ap, self_ap):
                # psum[o, p] = sum_e (W_a/8)[o,:] @ children[:, 8p+e]
                #            +  W_b[o,:] @ self[:, p]
                # children in natural node order -> group-sum the 8 siblings
                # on VectorE (1x-mode reduce, but DVE is idle), contract once.
                aggt = aggp.tile([C, PT], bf16, tag="agg", name="aggt")
                with nc.allow_low_precision(
                        reason="8-way sibling sum is fp32 internal on DVE; "
                               "bf16 rounding of the sum is within tolerance"):
                    nc.vector.reduce_sum(
                        aggt[:, :],
                        children_ap.rearrange("c (p e) -> c p e", e=8),
                        axis=AxX)
                nc.tensor.matmul(psum, w[wa], aggt[:, :],
                                 start=True, stop=False)
                nc.tensor.matmul(psum, w[wb], self_ap,
                                 start=False, stop=True)

            def sage_pe(psum, wa, wb, children_ap, self_ap):
                # children de-interleaved on host: col e*PT + p = child e of
                # parent p -> 8 accumulating matmuls, each rhs contiguous.
                for e in range(8):
                    nc.tensor.matmul(psum, w[wa],
                                     children_ap[:, PT * e: PT * (e + 1)],
                                     start=(e == 0), stop=False)
                nc.tensor.matmul(psum, w[wb], self_ap,
                                 start=False, stop=True)

            n_t = NPAR1 // PT                    # 9 outer tiles
            for t in range(n_t):
                # layer-0 tile for parents [512t, 512(t+1)) (hop0+hop1):
                # children are xA cols [512+4096t, ...) in natural order.
                ps0 = pp.tile([C, PT], f32, tag="ps")
                sage_dve(ps0, "w1a", "w1b",
                         xAt[:, S + 8 * PT * t: S + 8 * PT * (t + 1)],
                         xAt[:, PT * t: PT * (t + 1)])
                nc.scalar.activation(h1self[:, PT * t: PT * (t + 1)], ps0,
                                     Relu, bias=bt["b1"])

                if t == 0:
                    # parents [512, 4608) land in h1self via tiles t=1..8;
                    # layer-1 tile 0 is emitted after the loop from h1self.
                    continue

                # 8 layer-0 tiles for hop2 parents [512+4096t, 512+4096(t+1))
                # children ⊂ hop3: fp8, host de-interleaved per 512-group;
                # one 4 MiB DMA per t covers all 8 u-subtiles.
                X3t = dpool.tile([C, 8 * 8 * PT], fp8, tag="X3")
                nc.sync.dma_start(
                    X3t[:, :], x3[:, N3 // 8 * (t - 1): N3 // 8 * t])
                h1tmp = hpool.tile([C, 8 * PT], bf16, tag="h1tmp")
                for u in range(8):
                    psu = pp.tile([C, PT], f32, tag="ps")
                    sage_pe(psu, "w1a", "w1b",
                            X3t[:, 8 * PT * u: 8 * PT * (u + 1)],
                            xAt[:, S + 8 * PT * t + PT * u:
                                S + 8 * PT * t + PT * (u + 1)])
                    nc.scalar.activation(h1tmp[:, PT * u: PT * (u + 1)], psu,
                                         Relu, bias=bt["b1"])

                # layer-1 tile for parents [512t, 512(t+1)) -> h2
                ps1 = pp.tile([C, PT], f32, tag="ps")
                sage_dve(ps1, "w2a", "w2b", h1tmp[:, :],
                         h1self[:, PT * t: PT * (t + 1)])
                nc.scalar.activation(h2sb[:, PT * t: PT * (t + 1)], ps1,
                                     Relu, bias=bt["b2"])

            # layer-1 tile 0: children h1[512:4608) = h1self slice
            ps1z = pp.tile([C, PT], f32, tag="ps")
            sage_dve(ps1z, "w2a", "w2b", h1self[:, S:NPAR1],
                     h1self[:, 0:S])
            nc.scalar.activation(h2sb[:, 0:S], ps1z, Relu, bias=bt["b2"])

            # layer 2: parents [0, 512) aggregate h2[512:4608); rows
            # [512, 4608) have no in-edges (agg = 0) -> self term only.
            ps2 = pp.tile([C, PT], f32, tag="ps")
            sage_dve(ps2, "w3a", "w3b", h2sb[:, S:NPAR1], h2sb[:, 0:S])
            o0 = opool.tile([C, PT], bf16, tag="o")
            nc.scalar.activation(o0[:, :], ps2, Relu, bias=bt["b3"])
            nc.sync.dma_start(out[:, 0:S], o0[:, :])
            for t in range(1, n_t):
                psn = pp.tile([C, PT], f32, tag="ps")
                nc.tensor.matmul(psn, w["w3b"],
                                 h2sb[:, PT * t: PT * (t + 1)],
                                 start=True, stop=True)
                on = opool.tile([C, PT], bf16, tag="o")
                nc.scalar.activation(on[:, :], psn, Relu, bias=bt["b3"])
                nc.sync.dma_start(out[:, PT * t: PT * (t + 1)], on[:, :])

    nc.compile()
    return nc


def _get_bass():
    if "k" not in _BASS_CACHE:
        _BASS_CACHE["k"] = _build_bass()
    return _BASS_CACHE["k"]


def _edge_is_tree(edge):
    if edge.shape != (2, E_FULL):
        return False
    ar = np.arange(E_FULL, dtype=np.int64)
    return (np.array_equal(edge[0], (B + ar).astype(np.int32))
            and np.array_equal(edge[1], (ar // 8).astype(np.int32)))


def _fallback(x, edge, W1, b1, W2, b2, W3, b3):
    # General (structure-agnostic) CPU implementation; only used if the
    # inputs are not the fanout-8 tree this kernel is specialized for.
    sizes = [(N_FULL, E_FULL), (299008, 294912), (36864, 32768)]
    params = [(W1, b1), (W2, b2), (W3, b3)]
    x = x.astype(np.float32)
    for (n, e), (Wl, bl) in zip(sizes, params):
        src = edge[0, :e].astype(np.int64)
        dst = edge[1, :e].astype(np.int64)
        x = x[:n]
        agg = np.zeros((n, x.shape[1]), np.float32)
        np.add.at(agg, dst, x[src])
        deg = np.bincount(dst, minlength=n).astype(np.float32)
        agg /= np.maximum(deg, 1.0)[:, None]
        x = np.maximum(np.concatenate([agg, x], axis=1) @ Wl.T + bl, 0.0)
    return x


def kernel(**inputs):
    global LAST_RESULT
    import ml_dtypes

    x = np.asarray(inputs["x"])
    edge = np.asarray(inputs["edge"])
    W = [np.asarray(inputs[k], dtype=np.float32) for k in ("W1", "W2", "W3")]
    bias = [np.asarray(inputs[k], dtype=np.float32) for k in ("b1", "b2", "b3")]

    if x.shape != (N_FULL, C) or not _edge_is_tree(edge):
        return _fallback(x, edge, W[0], bias[0], W[1], bias[1], W[2], bias[2])

    from concourse.bass_utils import run_bass_kernel_spmd

    bf = ml_dtypes.bfloat16
    f8 = ml_dtypes.float8_e4m3fn          # bit-compatible with TRN e4m3 < 240
    x = np.ascontiguousarray(x, dtype=np.float32)

    wblocks = []
    for li in range(3):
        wblocks.append((W[li][:, :C] / 8.0).T)     # agg part, mean folded in
        wblocks.append(W[li][:, C:].T)             # self part
    wconsts = np.ascontiguousarray(np.concatenate(wblocks, axis=1)).astype(bf)
    bconsts = np.ascontiguousarray(np.stack(bias, axis=1))      # [128, 3] f32

    in_maps = []
    for c in range(N_CORES):
        xloc = [x[OFF[h] + BLK[h] * c: OFF[h] + BLK[h] * (c + 1)]
                for h in range(4)]
        xAc = np.ascontiguousarray(np.concatenate(xloc[:3], axis=0).T).astype(bf)
        # de-interleave hop3 per 512-parent group: within each 4096-row
        # chunk, row e*512 + p  <-  child e of parent p (old row 8p + e)
        x3 = xloc[3].reshape(-1, PT, 8, C).transpose(0, 2, 1, 3).reshape(-1, C)
        x3c = np.ascontiguousarray(x3.T).astype(f8)
        in_maps.append({"xA": xAc, "x3": x3c,
                        "wconsts": wconsts, "bconsts": bconsts})

    nc = _get_bass()
    res = run_bass_kernel_spmd(nc, in_maps, list(range(N_CORES)), trace=TRACE)
    LAST_RESULT = res

    out = np.empty((OUT_ROWS, C), np.float32)
    for c in range(N_CORES):
        oc = np.asarray(res.results[c]["out"]).astype(np.float32)
        out[S * c: S * (c + 1)] = oc[:, :S].T
        out[B + 8 * S * c: B + 8 * S * (c + 1)] = oc[:, S:].T
    return out


# revision 7
# speedup vs baseline: 4.5844x; 1.9984x over previous
"""Trainium2 Bass kernel for 3-layer CuGraphSAGE on a fanout-8 sampled tree.

The sampled graph produced by fanout-based neighbor sampling is a forest of
B=4096 independent trees (children of parent p are rows [4096+8p, 4096+8p+8)).
We shard by seed block: core c gets 512 seeds plus their full 3-hop subtrees
(4 contiguous row blocks of x, exactly 1/8 of all rows, zero halo).

Per-core pipeline (all activations channel-major [128ch, rows] so the matmul
contraction dim is always the partition dim — no transposes on device):
  hop3 (87.5% of bytes, only ever aggregated) streams as fp8_e4m3 — the
  8-way mean dilutes its 3.6% quantization RMS to ~1e-3 of the final output.
  It is de-interleaved per 512-parent group on the host (col e*512+p =
  child e of parent p), so the mean-aggregation is accumulating matmuls
  with CONTIGUOUS fp8 rhs slices (stride-8 APs stream ~5x slower on the
  PE); with GNN_DR=1 pairs of sibling planes go through fp8 DoubleRow
  matmuls (2 MACs/cell/cycle), halving PE streaming time.
  hops 0-2 (37376 rows) load as bf16 (one 1.2 MiB + eight 1 MiB chunks)
  serving all self terms and the hop1/hop2 aggregations; those small
  aggregations plus layers 1-2 run as VectorE reduce_sum (DVE is
  otherwise idle) followed by one matmul.  Instructions are emitted in a
  software-pipelined order so the DVE reductions and their dependent
  matmuls hide inside the big fp8 u-blocks.  The 1/8 mean is folded into
  the aggregation weight (via the activation scale on the DoubleRow
  path); bias+ReLU on ScalarE evicts PSUM->SBUF in bf16. h1/h2 live
  entirely in SBUF; the bf16 output is upcast to fp32 on the host.
"""

import os
import numpy as np

# ---------------------------------------------------------------- constants
N_CORES = 8
C = 128                       # channels
B = 4096                      # seeds
S = B // N_CORES              # 512 seeds per core
BLK = [512, 4096, 32768, 262144]          # per-core rows per hop
OFF = [0, 4096, 36864, 299008]            # global start row of each hop block
NPAR0 = BLK[0] + BLK[1] + BLK[2]          # 37376 local layer-0 parents
NPAR1 = BLK[0] + BLK[1]                   # 4608 local layer-1 parents
N3 = BLK[3]                               # 262144 local hop3 rows
PT = 512                                  # parents per PSUM tile
N_FULL = 2396160
E_FULL = 2392064
OUT_ROWS = 36864
DR_SCALE = 16.0               # fp8 DoubleRow agg-weight scale (see kernel())

TRACE = os.environ.get("GNN_TRACE", "0") == "1"
DR = os.environ.get("GNN_DR", "1") == "1"
LAST_RESULT = None

_BASS_CACHE = {}


def _build_bass(dr):
    import concourse.mybir as mybir
    from concourse import bacc
    from concourse.tile import TileContext

    bf16 = mybir.dt.bfloat16
    fp8 = mybir.dt.float8e4
    f32 = mybir.dt.float32
    Relu = mybir.ActivationFunctionType.Relu
    AxX = mybir.AxisListType.X
    DRow = mybir.MatmulPerfMode.DoubleRow

    # Bacc (not raw Bass): its compile() pipeline splits multi-sem sync
    # waits into event semaphores — TRN2 allows at most 1 wait/instruction.
    nc = bacc.Bacc()
    xA = nc.dram_tensor("xA", [C, NPAR0], bf16, kind="ExternalInput")
    x3 = nc.dram_tensor("x3", [C, N3], fp8, kind="ExternalInput")
    # all six 128x128 weight blocks packed into one tensor -> one DMA ->
    # one semaphore lane (per-instruction sync-wait slots are scarce)
    wconsts = nc.dram_tensor("wconsts", [C, 7 * C], bf16, kind="ExternalInput")
    wdr = nc.dram_tensor("wdr", [C, 2 * C], fp8, kind="ExternalInput")
    bconsts = nc.dram_tensor("bconsts", [C, 3], f32, kind="ExternalInput")
    out = nc.dram_tensor("out", [C, NPAR1], bf16, kind="ExternalOutput")
    WIDX = {k: i for i, k in
            enumerate(("w1a", "w1b", "w2a", "w2b", "w3a", "w3b", "w1bs"))}

    with TileContext(nc) as tc:
        with tc.tile_pool(name="const", bufs=1) as constp, \
             tc.tile_pool(name="keep", bufs=1) as keepp, \
             tc.tile_pool(name="dbuf", bufs=2) as dpool, \
             tc.tile_pool(name="hbuf", bufs=2) as hpool, \
             tc.tile_pool(name="obuf", bufs=2) as opool, \
             tc.tile_pool(name="a0buf", bufs=2) as a0p, \
             tc.tile_pool(name="a1buf", bufs=2) as a1p, \
             tc.tile_pool(name="ps", bufs=6, space="PSUM") as pp:

            wtile = constp.tile([C, 7 * C], bf16, name="wtile")
            nc.sync.dma_start(wtile[:, :], wconsts[:, :])
            wdrt = constp.tile([C, 2 * C], fp8, name="wdrt")
            nc.sync.dma_start(wdrt[:, :], wdr[:, :])
            btile = constp.tile([C, 3], f32, name="btile")
            nc.sync.dma_start(btile[:, :], bconsts[:, :])
            w = {k: wtile[:, C * i: C * (i + 1)] for k, i in WIDX.items()}
            bt = {f"b{i+1}": btile[:, i: i + 1] for i in range(3)}

            # hop0+hop1 (self + seed children), then hop2 in 8 per-block
            # chunks so the first fp8 u-block starts after ~6 MB, not 10.
            xA0 = keepp.tile([C, NPAR1], bf16, tag="xA0")
            nc.sync.dma_start(xA0[:, :], xA[:, 0:NPAR1])
            xH2 = []
            for t in range(1, 9):
                xh = keepp.tile([C, 8 * PT], bf16, tag=f"xh2_{t}",
                                name=f"xh2_{t}")
                xH2.append(xh)
            h1self = keepp.tile([C, NPAR1], bf16, tag="h1self")
            h2sb = keepp.tile([C, NPAR1], bf16, tag="h2sb")
            l2agg = keepp.tile([C, PT], bf16, tag="l2agg")

            def red8(dst_ap, children_ap):
                # dst[c, p] = sum_e children[c, 8p+e]  (natural node order)
                with nc.allow_low_precision(
                        reason="8-way sibling sum is fp32 internal on DVE; "
                               "bf16 rounding of the sum is within tolerance"):
                    nc.vector.reduce_sum(
                        dst_ap,
                        children_ap.rearrange("c (p e) -> c p e", e=8),
                        axis=AxX)

            def l0red(k):
                aggt = a0p.tile([C, PT], bf16, tag="agg0", name="aggt")
                ch = xA0[:, S:NPAR1] if k == 0 else xH2[k - 1][:, :]
                red8(aggt[:, :], ch)
                return aggt

            def l0mm(k, aggt):
                ps0 = pp.tile([C, PT], f32, tag="ps")
                nc.tensor.matmul(ps0, w["w1a"], aggt[:, :],
                                 start=True, stop=False)
                nc.tensor.matmul(ps0, w["w1b"], xA0[:, PT * k: PT * (k + 1)],
                                 start=False, stop=True)
                nc.scalar.activation(h1self[:, PT * k: PT * (k + 1)], ps0,
                                     Relu, bias=bt["b1"])

            def l1mm(t, aggt):
                # layer-1 tile for parents [512t, 512(t+1)) -> h2, plus the
                # (agg-free) layer-2 self-only output for the same columns.
                ps1 = pp.tile([C, PT], f32, tag="ps")
                nc.tensor.matmul(ps1, w["w2a"], aggt[:, :],
                                 start=True, stop=False)
                nc.tensor.matmul(ps1, w["w2b"],
                                 h1self[:, PT * t: PT * (t + 1)],
                                 start=False, stop=True)
                nc.scalar.activation(h2sb[:, PT * t: PT * (t + 1)], ps1,
                                     Relu, bias=bt["b2"])
                if t > 0:
                    psn = pp.tile([C, PT], f32, tag="ps")
                    nc.tensor.matmul(psn, w["w3b"],
                                     h2sb[:, PT * t: PT * (t + 1)],
                                     start=True, stop=True)
                    on = opool.tile([C, PT], bf16, tag="o")
                    nc.scalar.activation(on[:, :], psn, Relu, bias=bt["b3"])
                    nc.sync.dma_start(out[:, PT * t: PT * (t + 1)], on[:, :])

            # ---------------- software-pipelined main loop ----------------
            l0_pending = {}      # k -> aggt awaiting its matmul
            l1_aggs = {}         # t -> per-block layer-1 agg tile
            for t in range(1, 9):
                X3t = dpool.tile([C, 8 * 8 * PT], fp8, tag="X3")
                nc.sync.dma_start(
                    X3t[:, :], x3[:, N3 // 8 * (t - 1): N3 // 8 * t])
                nc.sync.dma_start(
                    xH2[t - 1][:, :],
                    xA[:, S + 8 * PT * t: S + 8 * PT * (t + 1)])

                h1tmp = hpool.tile([C, 8 * PT], bf16, tag="h1tmp")
                agg1 = a1p.tile([C, PT], bf16, tag="agg1", name="agg1")
                l1_aggs[t] = agg1
                for u in range(8):
                    # --- the big fp8 aggregation for 512 hop2 parents ---
                    psu = pp.tile([C, PT], f32, tag="ps")
                    cb = 8 * PT * u
                    if dr:
                        for e in range(4):
                            rhs = X3t[:, cb + 2 * PT * e: cb + 2 * PT * (e + 1)]
                            nc.tensor.matmul(
                                psu, wdrt[:, :].rearrange("c (j m) -> c j m", j=2),
                                rhs.rearrange("c (j n) -> c j n", j=2),
                                start=(e == 0), stop=False, perf_mode=DRow)
                        nc.tensor.matmul(
                            psu, w["w1bs"],
                            xH2[t - 1][:, PT * u: PT * (u + 1)],
                            start=False, stop=True)
                        nc.scalar.activation(
                            h1tmp[:, PT * u: PT * (u + 1)], psu, Relu,
                            bias=bt["b1"], scale=1.0 / (8.0 * DR_SCALE))
                    else:
                        for e in range(8):
                            nc.tensor.matmul(
                                psu, w["w1a"],
                                X3t[:, cb + PT * e: cb + PT * (e + 1)],
                                start=(e == 0), stop=False)
                        nc.tensor.matmul(
                            psu, w["w1b"],
                            xH2[t - 1][:, PT * u: PT * (u + 1)],
                            start=False, stop=True)
                        nc.scalar.activation(
                            h1tmp[:, PT * u: PT * (u + 1)], psu, Relu,
                            bias=bt["b1"])
                    # --- layer-1 partial aggregation for these 64 parents ---
                    red8(agg1[:, 64 * u: 64 * (u + 1)],
                         h1tmp[:, PT * u: PT * (u + 1)])

                    # --- interleaved small work (deps satisfied earlier) ---
                    if u == 1:
                        if t == 1:
                            l0_pending[0] = l0red(0)
                        l0_pending[t] = l0red(t)
                    elif u == 3 and t >= 2:
                        l1mm(t - 1, l1_aggs.pop(t - 1))
                        if t == 8:
                            # layer-2 agg, parents 0..447 (children in h2
                            # tiles 1..7, all written by now)
                            red8(l2agg[:, 0:448], h2sb[:, S: S + 8 * 448])
                    elif u == 6:
                        if t == 1 and 0 in l0_pending:
                            l0mm(0, l0_pending.pop(0))
                        elif t in l0_pending:
                            l0mm(t, l0_pending.pop(t))
                    elif u == 7 and t == 1 and 1 in l0_pending:
                        l0mm(1, l0_pending.pop(1))

            # ---------------- tail ----------------
            # layer-1 tile 0 (children h1[512:4608) = h1self tiles 1..8)
            l1a0 = a1p.tile([C, PT], bf16, tag="agg1", name="l1a0")
            red8(l1a0[:, :], h1self[:, S:NPAR1])
            l1mm(8, l1_aggs.pop(8))
            # layer-2 agg, parents 448..511 (children in h2 tile 8)
            red8(l2agg[:, 448:512], h2sb[:, S + 8 * 448: NPAR1])
            l1mm(0, l1a0)
            # layer 2, parents [0, 512): full agg + self on h2 tile 0
            ps2 = pp.tile([C, PT], f32, tag="ps")
            nc.tensor.matmul(ps2, w["w3a"], l2agg[:, :],
                             start=True, stop=False)
            nc.tensor.matmul(ps2, w["w3b"], h2sb[:, 0:S],
                             start=False, stop=True)
            o0 = opool.tile([C, PT], bf16, tag="o")
            nc.scalar.activation(o0[:, :], ps2, Relu, bias=bt["b3"])
            nc.sync.dma_start(out[:, 0:S], o0[:, :])

    nc.compile()
    return nc


def _get_bass(dr):
    if dr not in _BASS_CACHE:
        _BASS_CACHE[dr] = _build_bass(dr)
    return _BASS_CACHE[dr]


def _edge_is_tree(edge):
    if edge.shape != (2, E_FULL):
        return False
    ar = np.arange(E_FULL, dtype=np.int64)
    return (np.array_equal(edge[0], (B + ar).astype(np.int32))
            and np.array_equal(edge[1], (ar // 8).astype(np.int32)))


def _fallback(x, edge, W1, b1, W2, b2, W3, b3):
    # General (structure-agnostic) CPU implementation; only used if the
    # inputs are not the fanout-8 tree this kernel is specialized for.
    sizes = [(N_FULL, E_FULL), (299008, 294912), (36864, 32768)]
    params = [(W1, b1), (W2, b2), (W3, b3)]
    x = x.astype(np.float32)
    for (n, e), (Wl, bl) in zip(sizes, params):
        src = edge[0, :e].astype(np.int64)
        dst = edge[1, :e].astype(np.int64)
        x = x[:n]
        agg = np.zeros((n, x.shape[1]), np.float32)
        np.add.at(agg, dst, x[src])
        deg = np.bincount(dst, minlength=n).astype(np.float32)
        agg /= np.maximum(deg, 1.0)[:, None]
        x = np.maximum(np.concatenate([agg, x], axis=1) @ Wl.T + bl, 0.0)
    return x


def kernel(**inputs):
    global LAST_RESULT
    import ml_dtypes

    x = np.asarray(inputs["x"])
    edge = np.asarray(inputs["edge"])
    W = [np.asarray(inputs[k], dtype=np.float32) for k in ("W1", "W2", "W3")]
    bias = [np.asarray(inputs[k], dtype=np.float32) for k in ("b1", "b2", "b3")]

    if x.shape != (N_FULL, C) or not _edge_is_tree(edge):
        return _fallback(x, edge, W[0], bias[0], W[1], bias[1], W[2], bias[2])

    from concourse.bass_utils import run_bass_kernel_spmd

    bf = ml_dtypes.bfloat16
    f8 = ml_dtypes.float8_e4m3fn          # bit-compatible with TRN e4m3 < 240
    x = np.ascontiguousarray(x, dtype=np.float32)

    wblocks = []
    for li in range(3):
        wblocks.append((W[li][:, :C] / 8.0).T)     # agg part, mean folded in
        wblocks.append(W[li][:, C:].T)             # self part
    # DoubleRow path: fp8 agg weight scaled by DR_SCALE to sit in e4m3's
    # normal range; the matching self weight is scaled by 8*DR_SCALE and the
    # whole PSUM is divided back by 8*DR_SCALE in the activation (ReLU is
    # positively homogeneous), which also restores the /8 of the mean.
    wblocks.append(W[0][:, C:].T * (8.0 * DR_SCALE))          # w1bs
    wconsts = np.ascontiguousarray(np.concatenate(wblocks, axis=1)).astype(bf)
    w1a_s = (W[0][:, :C].T * DR_SCALE).astype(f8)
    wdr = np.ascontiguousarray(np.concatenate([w1a_s, w1a_s], axis=1))
    bconsts = np.ascontiguousarray(np.stack(bias, axis=1))      # [128, 3] f32

    in_maps = []
    for c in range(N_CORES):
        xloc = [x[OFF[h] + BLK[h] * c: OFF[h] + BLK[h] * (c + 1)]
                for h in range(4)]
        xAc = np.ascontiguousarray(np.concatenate(xloc[:3], axis=0).T).astype(bf)
        # de-interleave hop3 per 512-parent group: within each 4096-row
        # chunk, row e*512 + p  <-  child e of parent p (old row 8p + e)
        x3 = xloc[3].reshape(-1, PT, 8, C).transpose(0, 2, 1, 3).reshape(-1, C)
        x3c = np.ascontiguousarray(x3.T).astype(f8)
        in_maps.append({"xA": xAc, "x3": x3c, "wconsts": wconsts,
                        "wdr": wdr, "bconsts": bconsts})

    nc = _get_bass(DR)
    res = run_bass_kernel_spmd(nc, in_maps, list(range(N_CORES)), trace=TRACE)
    LAST_RESULT = res

    out = np.empty((OUT_ROWS, C), np.float32)
    for c in range(N_CORES):
        oc = np.asarray(res.results[c]["out"]).astype(np.float32)
        out[S * c: S * (c + 1)] = oc[:, :S].T
        out[B + 8 * S * c: B + 8 * S * (c + 1)] = oc[:, S:].T
    return out
